# revision 91
# baseline (speedup 1.0000x reference)
"""Multi-head linear self-attention (ELU+1 feature map) — Trainium2 Bass kernel.

Reference computation (b=4, n=4096, f=768, h=12, d=64):
    q = phi(x@Wq + bq), k = phi(x@Wk + bk), v = x@Wv + bv   with phi = elu+1
    kv[h] = k[h].T @ v[h]  (sum over full sequence)
    ksum[h] = sum_n k[h]
    z = 1/(q . ksum);  out = concat_h(q[h] @ kv[h] * z) @ Wo + bo

Sharding: 8 cores = batch(4) x head-half(2). Each core gets one batch element
and a 6-head column-slice of Wq/Wk/Wv (+ the matching row-slice of Wo) and
produces a partial output [4096, 768]. Host unshard = sum of the two partials
per batch (row-parallel tensor parallelism) + bo.

Structure (all matmul operands fp16, PSUM fp32):
  A: K/V projections in [n, g] layout (lhsT = x.T chunks). phi(t+bk) =
     max(min(exp(t+bk),1), t+bk+1) in 2 DVE ops (stt psum-evict + fused
     min/max stt) + 1 ACT exp; bk comes from a host-broadcast tile — no bias
     matmul chunks. bv never reaches the device: its whole output
     contribution collapses to the constant bv@Wo (z*(q.ksum) = 1), folded
     into the host-side bias add. kv[h] += k.T @ [v | 1] per head-pair (the
     ones column yields ksum for free).
  boundary: kvT = transpose(kv) on the PE, then kvWo[h] = kv[h].T @ Wo rows
     — folds the per-head kv contraction into the output projection so the
     separate q@kv stage disappears. q(0)/q(1) are emitted around the stage-A
     drain and the boundary so the PE is never idle during the cross-engine
     eviction chains.
  B: Q.T = Wq.T @ x.T per 512-row chunk (ACT applies bq per-partition).
     z-denominators come from ONE matmul per (mo, chunk): B = mask64 * ksum
     (built at the boundary on gpsimd) is the mask-expanded denominator
     weight matrix, so den arrives already partition-expanded — no separate
     esel expansion matmuls. reciprocal_approx_fast per-mo (its cost is
     ~3.3 cyc per FREE element, partition count free), qz multiplies split
     DVE/gpsimd to keep DVE off the c2 chain.
  C: out = qz @ kvWo per 128-row chunk, interleaved into the B loop so the
     PE has fill work during the z latency chains; fp16 output.

z rides as z*2^12 in fp16 (keeps it in the normal range); the 2^-12 is folded
into the final eviction (mask64 carries the 2^-12). Startup: HW DMA
arbitration starves other queues whenever sync (Q1) has backlog, so the
early-critical chain (xt0, wk, xt1, xt2, bkb, wv) rides sync alone in exact
consumption order (~150GB/s single-queue; the startup is bandwidth-bound at
~2.3MB critical bytes); stage A opens with three K chunks so the wv arrival
hides behind K fill work. x.T arrives host-pre-blocked per 128-row chunk so
every DMA reads 1.5KB-contiguous runs; the weights arrive host-pre-blocked
[partition, chunk, cols] so each partition reads one 2.3-4.6KB contiguous run
instead of 768B rows (worth ~2us on the bandwidth-bound startup). The last
output chunk's evictions/DMAs fan out across DVE/ACT x sync/gpsimd so the
tail drains in parallel.
"""

from contextlib import ExitStack

import ml_dtypes
import numpy as np

import concourse.bass as bass
import concourse.mybir as mybir
import concourse.tile as tile
from concourse import bacc
from concourse.bass_utils import run_bass_kernel_spmd

FP = mybir.dt.float32
HF = mybir.dt.float16
ALU = mybir.AluOpType
ACTF = mybir.ActivationFunctionType

P = 128
R = 4096          # sequence rows per core (one full batch element)
F = 768           # input features
H = 6             # heads per core
D = 64            # head dim
G = H * D         # 384 output features per core
KO = F // P       # 6 input-feature chunks
MO = G // P       # 3 output-feature chunks
NCH = R // P      # 32 row chunks of 128
RC = 512          # stage-B/C row chunk
NRC = R // RC     # 8
ZSCALE = 4096.0   # 2^12: z survives fp16 exactly-scaled

N_CORES = 8


def build_nc():
    nc = bacc.Bacc("TRN2", target_bir_lowering=False, debug=False)

    # x arrives pre-transposed AND pre-blocked per 128-row chunk so every DMA
    # reads 1.5KB-contiguous runs from HBM (512B runs are descriptor-bound)
    xt_in = nc.dram_tensor("xt_in", [NCH, P, KO, P], HF, kind="ExternalInput").ap()
    # weights arrive host-pre-blocked [partition, ko/mo, cols]: each
    # partition's DMA then reads one 2.3-4.6KB contiguous run instead of
    # 768B rows — the weight transfers sit on the bandwidth-bound startup
    # critical path
    wq = nc.dram_tensor("wq", [P, KO, G], HF, kind="ExternalInput").ap()
    wk = nc.dram_tensor("wk", [P, KO, G], HF, kind="ExternalInput").ap()
    wv = nc.dram_tensor("wv", [P, KO, G], HF, kind="ExternalInput").ap()
    wo = nc.dram_tensor("wo", [P, MO, F], HF, kind="ExternalInput").ap()
    bq = nc.dram_tensor("bq", [G], FP, kind="ExternalInput").ap()
    # bk broadcast along partitions so the phi chain adds it on DVE. bv never
    # reaches the device: its whole output contribution collapses to the
    # constant bv@Wo (z*(q.ksum) = 1), which the host folds into the bo add.
    bkb = nc.dram_tensor("bkb", [P, G], HF, kind="ExternalInput").ap()
    # mask64[r, c] = 2^-12 if (r<64)==(c<64) else 0. B[:, mo, :] =
    # mask64 * ksum (per-partition scale) turns the z-denominator matmul into
    # one that lands the PER-PARTITION-EXPANDED denominator directly, killing
    # the separate esel-expansion matmuls (partition-dim broadcast isn't
    # allowed on compute engines). The 2^-12 keeps z*2^12 in fp16 range.
    mask64 = nc.dram_tensor("mask64", [P, P], HF, kind="ExternalInput").ap()
    eye = nc.dram_tensor("eye", [P, P], HF, kind="ExternalInput").ap()
    y = nc.dram_tensor("y", [R, F], HF, kind="ExternalOutput").ap()

    with tile.TileContext(nc) as tc, ExitStack() as ctx:
        singles = ctx.enter_context(tc.tile_pool(name="singles", bufs=1))
        wpool = ctx.enter_context(tc.tile_pool(name="wpool", bufs=4))

        # x.T loaded in row blocks so stage A can start after the first block;
        # resident through stage B.
        # Startup critical path = first matmul needs xt0 + wk; wk leads the
        # scalar queue (in halves: the first ko matmuls start on half one).
        # wv splits across sync+gpsimd so V-proj of chunk 0 isn't gated by
        # gpsimd's late cold start. The bulk xt blocks ride gpsimd (idle all
        # of stage A) so the sync engine keeps slack for the y-row DMAs.
        xt_pool = ctx.enter_context(tc.tile_pool(name="xt", bufs=1))
        xt = xt_pool.tile([P, KO, R], HF)

        def xt_block(eng, i):
            eng.dma_start(xt[:, :, i * P : (i + 1) * P], xt_in[i])

        wk_r = wk
        wv_r = wv
        wk_sb = wpool.tile([P, KO, G], HF, tag="w")
        wv_sb = wpool.tile([P, KO, G], HF, tag="w")

        # HW DMA arbitration starves the other queues whenever the sync
        # (Q1) queue has backlog, so the ENTIRE early-critical chain rides
        # sync in exact consumption order; only late-needed bulk (bq/wq/wo,
        # first used at the stage-A drain ~70us in) goes to gpsimd scraps.
        xt_block(nc.sync, 0)
        nc.sync.dma_start(wk_sb[:, 0:3, :], wk_r[:, 0:3, :])
        nc.sync.dma_start(wk_sb[:, 3:6, :], wk_r[:, 3:6, :])
        xt_block(nc.sync, 1)
        xt_block(nc.sync, 2)
        bkb_sb = singles.tile([P, G], HF, tag="bkb_sb")
        nc.sync.dma_start(bkb_sb, bkb)
        nc.sync.dma_start(wv_sb[:, 0:3, :], wv_r[:, 0:3, :])
        nc.sync.dma_start(wv_sb[:, 3:6, :], wv_r[:, 3:6, :])
        for i in range(3, NCH):
            xt_block(nc.sync, i)
        eye_sb = singles.tile([P, P], HF, tag="eye_sb")
        nc.sync.dma_start(eye_sb, eye)
        mask64_sb = singles.tile([P, P], HF, tag="mask64_sb")
        nc.sync.dma_start(mask64_sb, mask64)

        bq_col = singles.tile([P, MO], FP, tag="bq_col")
        nc.gpsimd.dma_start(bq_col, bq.rearrange("(mo p) -> p mo", p=P))
        # stage B/C weights preload during stage A (wpool bufs=4: no reuse dep)
        wq_sb = wpool.tile([P, KO, G], HF, tag="w")
        nc.gpsimd.dma_start(wq_sb, wq)
        wo_sb = wpool.tile([P, MO, F], HF, tag="w")
        nc.gpsimd.dma_start(wo_sb, wo)
        # bq1_col is computed right before emit_q(0) (stage-A drain): emitting
        # its DVE op here would block the whole DVE queue on the bq DMA
        bq1_col = singles.tile([P, MO], FP, tag="bq1_col")
        neg1_col = singles.tile([P, 1], FP, tag="neg1_col")
        nc.vector.memset(neg1_col, -1.0)

        # stage-A outputs that persist
        kvblk = [
            singles.tile([P, P], HF, tag=f"kvblk{p}", name=f"kvblk{p}")
            for p in range(MO)
        ]
        ksum_sb = singles.tile([P, MO], FP, tag="ksum_sb")
        bmat = singles.tile([P, MO, P], HF, tag="bmat")
        kvwo_sb = singles.tile([P, MO, F], HF, tag="kvwo_sb")

        # persistent vext buffers: the ones column (for ksum) is written once
        vext_bufs = [
            singles.tile([P, H, D + 1], HF, tag=f"vext{j}", name=f"vext{j}")
            for j in range(4)
        ]
        for j in range(4):
            nc.vector.memset(vext_bufs[j][:, :, D : D + 1], 1.0)

        # Q-stage infrastructure lives outside the stage scopes: q(0)/q(1) are
        # emitted around the stage-A drain and the kvWo boundary so the PE has
        # fill work while the cross-engine eviction chains run
        qt_pool = ctx.enter_context(tc.tile_pool(name="qt", bufs=1))
        qt = qt_pool.tile([P, MO, R], HF)
        qz_pool = ctx.enter_context(tc.tile_pool(name="qz", bufs=1))
        qz = qz_pool.tile([P, MO, R], HF)
        qp_pool = ctx.enter_context(tc.tile_pool(name="qp", bufs=2, space="PSUM"))
        qe_pool = ctx.enter_context(tc.tile_pool(name="qe", bufs=3))

        def emit_q_mo(rc, mo):
            rs = slice(rc * RC, (rc + 1) * RC)
            qps = qp_pool.tile([P, RC], FP, name="qps")
            for ko in range(KO):
                nc.tensor.matmul(
                    qps,
                    lhsT=wq_sb[:, ko, mo * P : (mo + 1) * P],
                    rhs=xt[:, ko, rs],
                    start=(ko == 0),
                    stop=(ko == KO - 1),
                )
            # psum evicted twice in parallel: exp on ACT, the t+1 linear part
            # on DVE (tensor_scalar with per-partition bq1) — frees the ACT
            # queue for the c2 evictions and releases the q psum earlier
            # ACT evicts psum twice (exp and the t+1 linear part) so the
            # remaining DVE ops run on fp16 SBUF operands only
            e = qe_pool.tile([P, RC], HF, name="qe")
            nc.scalar.activation(e, qps, ACTF.Exp, bias=bq_col[:, mo : mo + 1])
            t1 = qe_pool.tile([P, RC], HF, tag="qt1", name="qt1")
            nc.scalar.activation(
                t1, qps, ACTF.Identity, bias=bq1_col[:, mo : mo + 1]
            )
            nc.vector.scalar_tensor_tensor(
                qt[:, mo, rs], e, 1.0, t1, op0=ALU.min, op1=ALU.max
            )

        def emit_q(rc):
            for mo in range(MO):
                emit_q_mo(rc, mo)

        # ---------------- stage A: K, V, kv, ksum ---------------------------
        with ExitStack() as sctx:
            kp_pool = sctx.enter_context(tc.tile_pool(name="kp", bufs=2, space="PSUM"))
            vp_pool = sctx.enter_context(tc.tile_pool(name="vp", bufs=1, space="PSUM"))
            kv_pool = sctx.enter_context(tc.tile_pool(name="kvp", bufs=1, space="PSUM"))
            ksb_pool = sctx.enter_context(tc.tile_pool(name="ksb", bufs=4))
            tmp_pool = sctx.enter_context(tc.tile_pool(name="katmp", bufs=3))

            kv_ps = [
                kv_pool.tile([P, 2 * (D + 1)], FP, tag=f"kv{p}", name=f"kv{p}")
                for p in range(MO)
            ]

            # kv matmuls run one chunk behind K/V so the PE never waits on the
            # phi/bias eviction chain of the current chunk
            pend = []

            def emit_kv(i, ksb, vext):
                for p in range(MO):
                    rhs = vext[:, 2 * p : 2 * p + 2, :].rearrange("p h e -> p (h e)")
                    nc.tensor.matmul(
                        kv_ps[p],
                        lhsT=ksb[:, p * P : (p + 1) * P],
                        rhs=rhs,
                        start=(i == 0),
                        stop=(i == NCH - 1),
                    )

            def emit_kproj(i):
                kps = kp_pool.tile([P, G], FP)
                for ko in range(KO):
                    nc.tensor.matmul(
                        kps,
                        lhsT=xt[:, ko, i * P : (i + 1) * P],
                        rhs=wk_sb[:, ko, :],
                        start=(ko == 0),
                        stop=(ko == KO - 1),
                    )
                # phi(t+bk) = max(min(exp(t+bk), 1), t+bk+1). tb1 = t+bk+1 in
                # one DVE stt off the psum; exp un-does the +1 via its -1.0
                # bias; min and max run in the fast fp16 DVE mode
                tb1 = tmp_pool.tile([P, G], HF, tag="ktb", name="ktb")
                nc.vector.scalar_tensor_tensor(
                    tb1, kps, 1.0, bkb_sb, op0=ALU.add, op1=ALU.add
                )
                e = tmp_pool.tile([P, G], HF, tag="ke", name="ke")
                nc.scalar.activation(e, tb1, ACTF.Exp, bias=neg1_col)
                ksb = ksb_pool.tile([P, G], HF)
                nc.vector.scalar_tensor_tensor(
                    ksb, e, 1.0, tb1, op0=ALU.min, op1=ALU.max
                )
                return ksb

            def emit_vproj(i):
                # V projection (no bv: host folds bv@Wo into the bias add);
                # the psum eviction runs on the stage-A-idle ACT engine
                vps = vp_pool.tile([P, G], FP)
                for ko in range(KO):
                    nc.tensor.matmul(
                        vps,
                        lhsT=xt[:, ko, i * P : (i + 1) * P],
                        rhs=wv_sb[:, ko, :],
                        start=(ko == 0),
                        stop=(ko == KO - 1),
                    )
                vext = vext_bufs[i % 4]
                nc.scalar.copy(
                    vext[:, :, 0:D], vps.rearrange("p (h d) -> p h d", d=D)
                )
                return vext

            # chunks 0-2 run K,K,K,V,V,V: the K projections only need wk +
            # the first xt blocks, covering the wv arrival latency (wv is
            # last in the sync queue's critical chain)
            ksb012 = [emit_kproj(0), emit_kproj(1), emit_kproj(2)]
            for i in (0, 1, 2):
                pend.append((i, ksb012[i], emit_vproj(i)))
            for i in range(3, NCH):
                ksb = emit_kproj(i)
                vext = emit_vproj(i)
                pend.append((i, ksb, vext))
                if len(pend) > 2:
                    emit_kv(*pend.pop(0))
            # q(0) before the last kv matmuls: it fills the PE while the last
            # chunks' phi chains drain through DVE/ACT
            nc.vector.tensor_scalar(bq1_col, bq_col, 1.0, None, op0=ALU.add)
            emit_q(0)
            while pend:
                emit_kv(*pend.pop(0))

            # extract kv block-diagonals and ksum columns; the block copies go
            # on ACT (idle at the A/B boundary) so the PE transposes that wait
            # on them are not gated by the DVE backlog
            # ACT is the scarce engine across the drain/boundary window (it
            # still owes V-copies and the q(0)/q(1) eviction chains that gate
            # the q-psum recycle), so the extraction spreads over DVE and the
            # idle gpsimd instead
            for p in range(MO):
                nc.vector.memset(kvblk[p], 0.0)
                nc.vector.tensor_copy(kvblk[p][0:D, 0:D], kv_ps[p][0:D, 0:D])
                nc.vector.tensor_copy(
                    kvblk[p][D:P, D:P], kv_ps[p][D:P, D + 1 : D + 1 + D]
                )
                # the ones column (D) holds ksum for ALL 128 partitions of the
                # pair; B[:, p, :] = mask64 * ksum lands the head-expanded,
                # 2^-12-scaled z-denominator weights for the den matmul
                nc.vector.tensor_copy(ksum_sb[:, p : p + 1], kv_ps[p][:, D : D + 1])
            for p in range(MO):
                nc.gpsimd.tensor_scalar(
                    bmat[:, p, :], mask64_sb, ksum_sb[:, p : p + 1], None,
                    op0=ALU.mult,
                )

        # ------- boundary: kvWo[h] = kv[h].T @ Wo rows (fold kv into Wo) ----
        # q(1) splits around the boundary: mo0/mo1 first so their ACT
        # eviction chains (the q-psum recycle gate for q(2)) drain under the
        # boundary matmuls; mo2 lands after. kvwo evictions ride DVE so the
        # ACT queue stays clear for the q evictions.
        emit_q_mo(1, 0)
        emit_q_mo(1, 1)
        with ExitStack() as sctx:
            kvt_ps_pool = sctx.enter_context(
                tc.tile_pool(name="kvtp", bufs=2, space="PSUM")
            )
            kvw_ps_pool = sctx.enter_context(
                tc.tile_pool(name="kvwp", bufs=2, space="PSUM")
            )
            kvt_sb_pool = sctx.enter_context(tc.tile_pool(name="kvts", bufs=3))
            tps = []
            for p in range(MO):
                tp = kvt_ps_pool.tile([P, P], HF, tag="kvtp", name="kvtp")
                nc.tensor.transpose(tp, kvblk[p], eye_sb)
                tps.append(tp)
            # all boundary evictions ride DVE: ACT must start the q(1)
            # eviction chain ASAP (it gates the q-psum recycle for q(2))
            kvts = []
            for p in range(MO):
                kvt = kvt_sb_pool.tile([P, P], HF, tag="kvts", name="kvts")
                nc.vector.tensor_copy(kvt, tps[p])
                kvts.append(kvt)
            for p in range(MO):
                kw = kvw_ps_pool.tile([P, 1024], FP, name="kvwp")
                for hh in range(2):
                    nc.tensor.matmul(
                        kw[:, hh * 512 : hh * 512 + F // 2],
                        lhsT=kvts[p],
                        rhs=wo_sb[:, p, hh * (F // 2) : (hh + 1) * (F // 2)],
                        start=True,
                        stop=True,
                    )
                nc.vector.tensor_copy(
                    kvwo_sb[:, p, :].rearrange("p (hh f) -> p hh f", hh=2),
                    kw.rearrange("p (hh f) -> p hh f", hh=2)[:, :, 0 : F // 2],
                )
        emit_q_mo(1, 2)

        # ------- stages B (Q.T), z (den+recip), qz, C2: staggered per chunk -
        with ExitStack() as sctx:
            zp_pool = sctx.enter_context(tc.tile_pool(name="zp", bufs=1, space="PSUM"))
            op_pool = sctx.enter_context(tc.tile_pool(name="outp", bufs=3, space="PSUM"))
            zr_pool = sctx.enter_context(tc.tile_pool(name="zrp", bufs=2))
            out_pool = sctx.enter_context(tc.tile_pool(name="osb", bufs=4))

            def emit_den(rc):
                # den_mo[c, n] = sum_g bmat[g, mo, c] q[g, mo, n]: the mask in
                # bmat makes every output partition c carry its own head's
                # denominator * 2^-12 — already expanded for the qz multiply.
                # recip costs ~3.3 cyc per FREE element (partition count is
                # free), so it stays per-mo [128, RC]; the qz multiplies run
                # on the B-phase-idle gpsimd to keep DVE off the c2 chain.
                rs = slice(rc * RC, (rc + 1) * RC)
                # zr must be fp32: reciprocal_approx_* depends on the fp32
                # bit layout, and a separate fp16 cast would cost more than
                # the 2x 16-bit qz mode saves
                zr = zr_pool.tile([P, MO, RC], FP, tag="zr", name="zr")
                for mo in range(MO):
                    zps = zp_pool.tile([P, RC], FP, tag=f"zps{mo}", name="zps")
                    nc.tensor.matmul(
                        zps, lhsT=bmat[:, mo, :], rhs=qt[:, mo, rs],
                        start=True, stop=True,
                    )
                    nc.vector.reciprocal_approx_fast(zr[:, mo, :], zps)
                    # qz rides z*2^12-scaled; the 2^-12 is folded into the C2
                    # output eviction. gpsimd's tensor_tensor is ~1.5us/op,
                    # so rc 0/1 (whose qz gates c2(0) right at the B start)
                    # stay on DVE; steady state keeps mo0 (the first block c2
                    # reads) on DVE and offloads mo1/mo2 to the idle gpsimd.
                    if rc < 2 or mo == 0:
                        eng = nc.vector
                    else:
                        eng = nc.gpsimd
                    eng.tensor_tensor(
                        qz[:, mo, rs], qt[:, mo, rs], zr[:, mo, :], op=ALU.mult
                    )

            def emit_c2(rc, subs=(0, 1, 2, 3)):
                # dense output projection, interleaved into the B loop so the
                # PE has fill work during the z latency chains
                for sub in subs:
                    r0 = rc * RC + sub * P
                    osb = out_pool.tile([P, F], HF)
                    for hh in range(2):
                        o_ps = op_pool.tile([P, F // 2], FP, tag="op", name="ops")
                        for p in range(MO):
                            nc.tensor.matmul(
                                o_ps,
                                lhsT=qz[:, p, r0 : r0 + P],
                                rhs=kvwo_sb[:, p, hh * (F // 2) : (hh + 1) * (F // 2)],
                                start=(p == 0),
                                stop=(p == MO - 1),
                            )
                        # output arrives 2^12-scaled (z pre-scale); exact
                        # unscale split across DVE and ACT; the two DMA halves
                        # ride the two HWDGE queues so the tail drains fast
                        oseg = osb[:, hh * (F // 2) : (hh + 1) * (F // 2)]
                        if hh == 0:
                            with nc.allow_low_precision(reason="fp16 output"):
                                nc.vector.tensor_scalar(
                                    oseg, o_ps, 1.0 / ZSCALE, None, op0=ALU.mult
                                )
                            if rc == NRC - 1:
                                # last chunk: fire each half's DMA immediately;
                                # the last two subs fan out across the
                                # tail-idle engines so the eviction->issue->
                                # transfer chains run in parallel
                                eng = (nc.sync, nc.sync, nc.gpsimd, nc.gpsimd)[sub]
                                eng.dma_start(y[r0 : r0 + P, 0 : F // 2], oseg)
                        else:
                            nc.scalar.activation(
                                oseg, o_ps, ACTF.Identity, scale=1.0 / ZSCALE
                            )
                            if rc == NRC - 1:
                                # no DMA issues on the ACT engine here: they
                                # would wedge between the hh1 evictions and
                                # delay the final eviction chain
                                eng = (nc.gpsimd, nc.gpsimd, nc.sync, nc.sync)[sub]
                                eng.dma_start(y[r0 : r0 + P, F // 2 :], oseg)
                    # one full-row DMA on the C2-idle sync queue (the ACT
                    # queue is near its limit with the hh=1 evictions)
                    if rc != NRC - 1:
                        nc.sync.dma_start(y[r0 : r0 + P, :], osb)

            # den(0) right at the B-stage start: its recip+qz chains (~5us on
            # gpsimd) drain under q(2) so c2(0) doesn't wait on them
            emit_den(0)
            # per iteration: den(rc-1) woven between q(rc)'s psum groups (its
            # qt landed last iteration; the zp psum was freed by the previous
            # recip, and the recip+qz chains drain under the remaining
            # matmuls), then c2(rc-2) (its qz completed a full iteration ago)
            # c2 sits early in the iteration so its psum-freeing evictions
            # land early in the DVE/ACT queues (the op-psum recycle is the
            # tightest cross-engine chain in phase B)
            for rc in range(2, NRC):
                emit_q_mo(rc, 0)
                emit_q_mo(rc, 1)
                emit_den(rc - 1)
                emit_q_mo(rc, 2)
                emit_c2(rc - 2)
            # tail: den(7)'s recip+qz chain drains under c2(6)'s matmuls
            emit_c2(NRC - 2, subs=(0,))
            emit_den(NRC - 1)
            emit_c2(NRC - 2, subs=(1, 2, 3))
            emit_c2(NRC - 1)

    nc.compile()
    return nc


def make_in_maps(x, Wq, bq, Wk, bk, Wv, bv, Wo, bo):
    """Shard full inputs into the 8 per-core input maps."""
    f32 = lambda a: np.ascontiguousarray(np.asarray(a, dtype=np.float32))
    f16 = lambda a: np.ascontiguousarray(np.asarray(a).astype(np.float16))
    # pre-block weights [in/out, cols] -> [partition, chunk, cols] so each
    # partition's DMA reads one contiguous run
    blkw = lambda a: np.ascontiguousarray(
        np.asarray(a).astype(np.float16).reshape(KO, P, G).transpose(1, 0, 2)
    )
    blko = lambda a: np.ascontiguousarray(
        np.asarray(a).astype(np.float16).reshape(MO, P, F).transpose(1, 0, 2)
    )
    # bv's output contribution is exactly the constant bv@Wo (z*(q.ksum)=1),
    # so it folds into the host-side bias add instead of riding the device
    _BO["bo_eff"] = np.asarray(
        np.asarray(bo, np.float64) + np.asarray(bv, np.float64) @ np.asarray(Wo, np.float64),
        np.float32,
    )
    Wq, Wk, Wv, Wo = map(f16, (Wq, Wk, Wv, Wo))
    bq = f32(bq)
    bk16 = f16(bk)
    x = np.asarray(x)
    # [768, 4096] -> [NCH, P, KO, P] so each 128-row block DMA reads
    # 1.5KB-contiguous runs
    xT = [
        np.ascontiguousarray(
            f16(x[b]).T.reshape(KO, P, NCH, P).transpose(2, 1, 0, 3)
        )
        for b in range(N_CORES // 2)
    ]
    # mask64[r, c] = 2^-12 where r and c fall in the same head-half; scaled by
    # ksum on-device it becomes the expanded z-denominator weight matrix
    mask64 = np.zeros((P, P), dtype=np.float16)
    mask64[:D, :D] = 1.0 / ZSCALE
    mask64[D:, D:] = 1.0 / ZSCALE
    eye = np.eye(P, dtype=np.float16)
    in_maps = []
    for c in range(N_CORES):
        b, g = divmod(c, 2)
        sl = slice(g * G, (g + 1) * G)
        in_maps.append(
            {
                "xt_in": xT[b],
                "wq": blkw(Wq[:, sl]),
                "wk": blkw(Wk[:, sl]),
                "wv": blkw(Wv[:, sl]),
                "wo": blko(Wo[sl, :]),
                "bq": f32(bq[sl]),
                "bkb": np.ascontiguousarray(np.tile(bk16[sl], (P, 1))),
                "mask64": mask64,
                "eye": eye,
            }
        )
    return in_maps


_BO = {}


def unshard(core_outs):
    """Sum the two row-parallel partials per batch element; add bo + bv@Wo."""
    bo_eff = _BO["bo_eff"]
    return np.stack(
        [
            core_outs[2 * b].astype(np.float32)
            + core_outs[2 * b + 1].astype(np.float32)
            + bo_eff
            for b in range(N_CORES // 2)
        ]
    )


_NC_CACHE = {}


def get_nc():
    if "nc" not in _NC_CACHE:
        _NC_CACHE["nc"] = build_nc()
    return _NC_CACHE["nc"]


def run(inputs, trace=False, **kwargs):
    nc = get_nc()
    in_maps = make_in_maps(**inputs)
    res = run_bass_kernel_spmd(
        nc, in_maps, core_ids=list(range(N_CORES)), trace=trace, **kwargs
    )
    out = unshard([r["y"] for r in res.results])
    return out, res


def kernel(**inputs):
    out, _ = run(inputs, trace=False)
    return out



# revision 93
# speedup vs baseline: 1.0125x; 1.0125x over previous
"""Multi-head linear self-attention (ELU+1 feature map) — Trainium2 Bass kernel.

Reference computation (b=4, n=4096, f=768, h=12, d=64):
    q = phi(x@Wq + bq), k = phi(x@Wk + bk), v = x@Wv + bv   with phi = elu+1
    kv[h] = k[h].T @ v[h]  (sum over full sequence)
    ksum[h] = sum_n k[h]
    z = 1/(q . ksum);  out = concat_h(q[h] @ kv[h] * z) @ Wo + bo

Sharding: 8 cores = batch(4) x head-half(2). Each core gets one batch element
and a 6-head column-slice of Wq/Wk/Wv (+ the matching row-slice of Wo) and
produces a partial output [4096, 768]. Host unshard = sum of the two partials
per batch (row-parallel tensor parallelism) + bo.

Structure (all matmul operands fp16, PSUM fp32):
  A: K/V projections in [n, g] layout (lhsT = x.T chunks). phi(t+bk) =
     max(min(exp(t+bk),1), t+bk+1) in 2 DVE ops (stt psum-evict + fused
     min/max stt) + 1 ACT exp; bk comes from a host-broadcast tile — no bias
     matmul chunks. bv never reaches the device: its whole output
     contribution collapses to the constant bv@Wo (z*(q.ksum) = 1), folded
     into the host-side bias add. kv[h] += k.T @ [v | 1] per head-pair (the
     ones column yields ksum for free).
  boundary: kvT = transpose(kv) on the PE, then kvWo[h] = kv[h].T @ Wo rows
     — folds the per-head kv contraction into the output projection so the
     separate q@kv stage disappears. q(0)/q(1) are emitted around the stage-A
     drain and the boundary so the PE is never idle during the cross-engine
     eviction chains.
  B: Q.T = Wq.T @ x.T per 512-row chunk (ACT applies bq per-partition).
     z-denominators come from ONE matmul per (mo, chunk): B = mask64 * ksum
     (built at the boundary on gpsimd) is the mask-expanded denominator
     weight matrix, so den arrives already partition-expanded — no separate
     esel expansion matmuls. reciprocal_approx_fast per-mo (its cost is
     ~3.3 cyc per FREE element, partition count free), qz multiplies split
     DVE/gpsimd to keep DVE off the c2 chain.
  C: out = qz @ kvWo per 128-row chunk, interleaved into the B loop so the
     PE has fill work during the z latency chains; fp16 output.

z rides as z*2^12 in fp16 (keeps it in the normal range); the 2^-12 is folded
into the final eviction (mask64 carries the 2^-12). Startup: HW DMA
arbitration starves other queues whenever sync (Q1) has backlog, so the
early-critical chain (xt0, wk, xt1, xt2, bkb, wv) rides sync alone in exact
consumption order (~150GB/s single-queue; the startup is bandwidth-bound at
~2.3MB critical bytes); stage A opens with three K chunks so the wv arrival
hides behind K fill work. x.T arrives host-pre-blocked per 128-row chunk so
every DMA reads 1.5KB-contiguous runs; the weights arrive host-pre-blocked
[partition, chunk, cols] so each partition reads one 2.3-4.6KB contiguous run
instead of 768B rows (worth ~2us on the bandwidth-bound startup). The last
output chunk's evictions/DMAs fan out across DVE/ACT x sync/gpsimd so the
tail drains in parallel.
"""

from contextlib import ExitStack

import ml_dtypes
import numpy as np

import concourse.bass as bass
import concourse.mybir as mybir
import concourse.tile as tile
from concourse import bacc
from concourse.bass_utils import run_bass_kernel_spmd

FP = mybir.dt.float32
HF = mybir.dt.float16
ALU = mybir.AluOpType
ACTF = mybir.ActivationFunctionType

P = 128
R = 4096          # sequence rows per core (one full batch element)
F = 768           # input features
H = 6             # heads per core
D = 64            # head dim
G = H * D         # 384 output features per core
KO = F // P       # 6 input-feature chunks
MO = G // P       # 3 output-feature chunks
NCH = R // P      # 32 row chunks of 128
RC = 512          # stage-B/C row chunk
NRC = R // RC     # 8
ZSCALE = 4096.0   # 2^12: z survives fp16 exactly-scaled

N_CORES = 8


def build_nc():
    nc = bacc.Bacc("TRN2", target_bir_lowering=False, debug=False)

    # x arrives pre-transposed AND pre-blocked per 128-row chunk so every DMA
    # reads 1.5KB-contiguous runs from HBM (512B runs are descriptor-bound)
    xt_in = nc.dram_tensor("xt_in", [NCH, P, KO, P], HF, kind="ExternalInput").ap()
    # weights arrive host-pre-blocked [partition, ko/mo, cols]: each
    # partition's DMA then reads one 2.3-4.6KB contiguous run instead of
    # 768B rows — the weight transfers sit on the bandwidth-bound startup
    # critical path
    wq = nc.dram_tensor("wq", [P, KO, G], HF, kind="ExternalInput").ap()
    wk = nc.dram_tensor("wk", [P, KO, G], HF, kind="ExternalInput").ap()
    wv = nc.dram_tensor("wv", [P, KO, G], HF, kind="ExternalInput").ap()
    wo = nc.dram_tensor("wo", [P, MO, F], HF, kind="ExternalInput").ap()
    bq = nc.dram_tensor("bq", [G], FP, kind="ExternalInput").ap()
    # bk broadcast along partitions so the phi chain adds it on DVE. bv never
    # reaches the device: its whole output contribution collapses to the
    # constant bv@Wo (z*(q.ksum) = 1), which the host folds into the bo add.
    bkb = nc.dram_tensor("bkb", [P, G], HF, kind="ExternalInput").ap()
    # mask64[r, c] = 2^-12 if (r<64)==(c<64) else 0. B[:, mo, :] =
    # mask64 * ksum (per-partition scale) turns the z-denominator matmul into
    # one that lands the PER-PARTITION-EXPANDED denominator directly, killing
    # the separate esel-expansion matmuls (partition-dim broadcast isn't
    # allowed on compute engines). The 2^-12 keeps z*2^12 in fp16 range.
    mask64 = nc.dram_tensor("mask64", [P, P], HF, kind="ExternalInput").ap()
    eye = nc.dram_tensor("eye", [P, P], HF, kind="ExternalInput").ap()
    y = nc.dram_tensor("y", [R, F], HF, kind="ExternalOutput").ap()

    with tile.TileContext(nc) as tc, ExitStack() as ctx:
        singles = ctx.enter_context(tc.tile_pool(name="singles", bufs=1))
        wpool = ctx.enter_context(tc.tile_pool(name="wpool", bufs=4))

        # x.T loaded in row blocks so stage A can start after the first block;
        # resident through stage B.
        # Startup critical path = first matmul needs xt0 + wk; wk leads the
        # scalar queue (in halves: the first ko matmuls start on half one).
        # wv splits across sync+gpsimd so V-proj of chunk 0 isn't gated by
        # gpsimd's late cold start. The bulk xt blocks ride gpsimd (idle all
        # of stage A) so the sync engine keeps slack for the y-row DMAs.
        xt_pool = ctx.enter_context(tc.tile_pool(name="xt", bufs=1))
        xt = xt_pool.tile([P, KO, R], HF)

        def xt_block(eng, i):
            eng.dma_start(xt[:, :, i * P : (i + 1) * P], xt_in[i])

        wk_r = wk
        wv_r = wv
        wk_sb = wpool.tile([P, KO, G], HF, tag="w")
        wv_sb = wpool.tile([P, KO, G], HF, tag="w")

        # HW DMA arbitration starves the other queues whenever the sync
        # (Q1) queue has backlog, so the ENTIRE early-critical chain rides
        # sync in exact consumption order; only late-needed bulk (bq/wq/wo,
        # first used at the stage-A drain ~70us in) goes to gpsimd scraps.
        xt_block(nc.sync, 0)
        nc.sync.dma_start(wk_sb[:, 0:3, :], wk_r[:, 0:3, :])
        nc.sync.dma_start(wk_sb[:, 3:6, :], wk_r[:, 3:6, :])
        xt_block(nc.sync, 1)
        xt_block(nc.sync, 2)
        bkb_sb = singles.tile([P, G], HF, tag="bkb_sb")
        nc.sync.dma_start(bkb_sb, bkb)
        nc.sync.dma_start(wv_sb[:, 0:3, :], wv_r[:, 0:3, :])
        nc.sync.dma_start(wv_sb[:, 3:6, :], wv_r[:, 3:6, :])
        for i in range(3, NCH):
            xt_block(nc.sync, i)
        eye_sb = singles.tile([P, P], HF, tag="eye_sb")
        nc.sync.dma_start(eye_sb, eye)
        mask64_sb = singles.tile([P, P], HF, tag="mask64_sb")
        nc.sync.dma_start(mask64_sb, mask64)

        bq_col = singles.tile([P, MO], FP, tag="bq_col")
        nc.gpsimd.dma_start(bq_col, bq.rearrange("(mo p) -> p mo", p=P))
        # stage B/C weights preload during stage A (wpool bufs=4: no reuse dep)
        wq_sb = wpool.tile([P, KO, G], HF, tag="w")
        nc.gpsimd.dma_start(wq_sb, wq)
        wo_sb = wpool.tile([P, MO, F], HF, tag="w")
        nc.gpsimd.dma_start(wo_sb, wo)
        # bq1_col is computed right before emit_q(0) (stage-A drain): emitting
        # its DVE op here would block the whole DVE queue on the bq DMA
        bq1_col = singles.tile([P, MO], FP, tag="bq1_col")
        neg1_col = singles.tile([P, 1], FP, tag="neg1_col")
        nc.vector.memset(neg1_col, -1.0)

        # stage-A outputs that persist
        kvblk = [
            singles.tile([P, P], HF, tag=f"kvblk{p}", name=f"kvblk{p}")
            for p in range(MO)
        ]
        ksum_sb = singles.tile([P, MO], FP, tag="ksum_sb")
        bmat = singles.tile([P, MO, P], HF, tag="bmat")
        kvwo_sb = singles.tile([P, MO, F], HF, tag="kvwo_sb")

        # persistent vext buffers: the ones column (for ksum) is written once
        vext_bufs = [
            singles.tile([P, H, D + 1], HF, tag=f"vext{j}", name=f"vext{j}")
            for j in range(4)
        ]
        for j in range(4):
            nc.vector.memset(vext_bufs[j][:, :, D : D + 1], 1.0)

        # Q-stage infrastructure lives outside the stage scopes: q(0)/q(1) are
        # emitted around the stage-A drain and the kvWo boundary so the PE has
        # fill work while the cross-engine eviction chains run
        qt_pool = ctx.enter_context(tc.tile_pool(name="qt", bufs=1))
        qt = qt_pool.tile([P, MO, R], HF)
        qz_pool = ctx.enter_context(tc.tile_pool(name="qz", bufs=1))
        qz = qz_pool.tile([P, MO, R], HF)
        qp_pool = ctx.enter_context(tc.tile_pool(name="qp", bufs=2, space="PSUM"))
        qe_pool = ctx.enter_context(tc.tile_pool(name="qe", bufs=3))

        def emit_q_mo(rc, mo):
            rs = slice(rc * RC, (rc + 1) * RC)
            qps = qp_pool.tile([P, RC], FP, name="qps")
            for ko in range(KO):
                nc.tensor.matmul(
                    qps,
                    lhsT=wq_sb[:, ko, mo * P : (mo + 1) * P],
                    rhs=xt[:, ko, rs],
                    start=(ko == 0),
                    stop=(ko == KO - 1),
                )
            # psum evicted twice in parallel: exp on ACT, the t+1 linear part
            # on DVE (tensor_scalar with per-partition bq1) — frees the ACT
            # queue for the c2 evictions and releases the q psum earlier
            # ACT evicts psum twice (exp and the t+1 linear part) so the
            # remaining DVE ops run on fp16 SBUF operands only
            e = qe_pool.tile([P, RC], HF, name="qe")
            nc.scalar.activation(e, qps, ACTF.Exp, bias=bq_col[:, mo : mo + 1])
            t1 = qe_pool.tile([P, RC], HF, tag="qt1", name="qt1")
            nc.scalar.activation(
                t1, qps, ACTF.Identity, bias=bq1_col[:, mo : mo + 1]
            )
            nc.vector.scalar_tensor_tensor(
                qt[:, mo, rs], e, 1.0, t1, op0=ALU.min, op1=ALU.max
            )

        def emit_q(rc):
            for mo in range(MO):
                emit_q_mo(rc, mo)

        # ---------------- stage A: K, V, kv, ksum ---------------------------
        with ExitStack() as sctx:
            kp_pool = sctx.enter_context(tc.tile_pool(name="kp", bufs=2, space="PSUM"))
            vp_pool = sctx.enter_context(tc.tile_pool(name="vp", bufs=1, space="PSUM"))
            kv_pool = sctx.enter_context(tc.tile_pool(name="kvp", bufs=1, space="PSUM"))
            ksb_pool = sctx.enter_context(tc.tile_pool(name="ksb", bufs=4))
            tmp_pool = sctx.enter_context(tc.tile_pool(name="katmp", bufs=3))

            kv_ps = [
                kv_pool.tile([P, 2 * (D + 1)], FP, tag=f"kv{p}", name=f"kv{p}")
                for p in range(MO)
            ]

            # kv matmuls run one chunk behind K/V so the PE never waits on the
            # phi/bias eviction chain of the current chunk
            pend = []

            def emit_kv(i, ksb, vext):
                for p in range(MO):
                    rhs = vext[:, 2 * p : 2 * p + 2, :].rearrange("p h e -> p (h e)")
                    nc.tensor.matmul(
                        kv_ps[p],
                        lhsT=ksb[:, p * P : (p + 1) * P],
                        rhs=rhs,
                        start=(i == 0),
                        stop=(i == NCH - 1),
                    )

            def emit_kproj(i):
                kps = kp_pool.tile([P, G], FP)
                for ko in range(KO):
                    nc.tensor.matmul(
                        kps,
                        lhsT=xt[:, ko, i * P : (i + 1) * P],
                        rhs=wk_sb[:, ko, :],
                        start=(ko == 0),
                        stop=(ko == KO - 1),
                    )
                # phi(t+bk) = max(min(exp(t+bk), 1), t+bk+1). tb1 = t+bk+1 in
                # one DVE stt off the psum; exp un-does the +1 via its -1.0
                # bias; min and max run in the fast fp16 DVE mode
                tb1 = tmp_pool.tile([P, G], HF, tag="ktb", name="ktb")
                nc.vector.scalar_tensor_tensor(
                    tb1, kps, 1.0, bkb_sb, op0=ALU.add, op1=ALU.add
                )
                e = tmp_pool.tile([P, G], HF, tag="ke", name="ke")
                nc.scalar.activation(e, tb1, ACTF.Exp, bias=neg1_col)
                ksb = ksb_pool.tile([P, G], HF)
                nc.vector.scalar_tensor_tensor(
                    ksb, e, 1.0, tb1, op0=ALU.min, op1=ALU.max
                )
                return ksb

            def emit_vproj(i):
                # V projection (no bv: host folds bv@Wo into the bias add);
                # the psum eviction runs on the stage-A-idle ACT engine
                vps = vp_pool.tile([P, G], FP)
                for ko in range(KO):
                    nc.tensor.matmul(
                        vps,
                        lhsT=xt[:, ko, i * P : (i + 1) * P],
                        rhs=wv_sb[:, ko, :],
                        start=(ko == 0),
                        stop=(ko == KO - 1),
                    )
                vext = vext_bufs[i % 4]
                nc.scalar.copy(
                    vext[:, :, 0:D], vps.rearrange("p (h d) -> p h d", d=D)
                )
                return vext

            # chunks 0-2 run K,K,K,V,V,V: the K projections only need wk +
            # the first xt blocks, covering the wv arrival latency (wv is
            # last in the sync queue's critical chain)
            ksb012 = [emit_kproj(0), emit_kproj(1), emit_kproj(2)]
            for i in (0, 1, 2):
                pend.append((i, ksb012[i], emit_vproj(i)))
            for i in range(3, NCH):
                ksb = emit_kproj(i)
                vext = emit_vproj(i)
                pend.append((i, ksb, vext))
                if len(pend) > 2:
                    emit_kv(*pend.pop(0))
            # q(0) before the last kv matmuls: it fills the PE while the last
            # chunks' phi chains drain through DVE/ACT
            nc.vector.tensor_scalar(bq1_col, bq_col, 1.0, None, op0=ALU.add)
            emit_q(0)
            while pend:
                emit_kv(*pend.pop(0))

            # extract kv block-diagonals and ksum columns; the block copies go
            # on ACT (idle at the A/B boundary) so the PE transposes that wait
            # on them are not gated by the DVE backlog
            # ACT is the scarce engine across the drain/boundary window (it
            # still owes V-copies and the q(0)/q(1) eviction chains that gate
            # the q-psum recycle), so the extraction spreads over DVE and the
            # idle gpsimd instead
            for p in range(MO):
                nc.vector.memset(kvblk[p], 0.0)
                nc.vector.tensor_copy(kvblk[p][0:D, 0:D], kv_ps[p][0:D, 0:D])
                nc.vector.tensor_copy(
                    kvblk[p][D:P, D:P], kv_ps[p][D:P, D + 1 : D + 1 + D]
                )
                # the ones column (D) holds ksum for ALL 128 partitions of the
                # pair; B[:, p, :] = mask64 * ksum lands the head-expanded,
                # 2^-12-scaled z-denominator weights for the den matmul
                nc.vector.tensor_copy(ksum_sb[:, p : p + 1], kv_ps[p][:, D : D + 1])
            for p in range(MO):
                nc.gpsimd.tensor_scalar(
                    bmat[:, p, :], mask64_sb, ksum_sb[:, p : p + 1], None,
                    op0=ALU.mult,
                )

        # ------- boundary: kvWo[h] = kv[h].T @ Wo rows (fold kv into Wo) ----
        # q(1) splits around the boundary: mo0/mo1 first so their ACT
        # eviction chains (the q-psum recycle gate for q(2)) drain under the
        # boundary matmuls; mo2 lands after. kvwo evictions ride DVE so the
        # ACT queue stays clear for the q evictions.
        emit_q_mo(1, 0)
        emit_q_mo(1, 1)
        with ExitStack() as sctx:
            kvt_ps_pool = sctx.enter_context(
                tc.tile_pool(name="kvtp", bufs=2, space="PSUM")
            )
            kvw_ps_pool = sctx.enter_context(
                tc.tile_pool(name="kvwp", bufs=2, space="PSUM")
            )
            kvt_sb_pool = sctx.enter_context(tc.tile_pool(name="kvts", bufs=3))
            tps = []
            for p in range(MO):
                tp = kvt_ps_pool.tile([P, P], HF, tag="kvtp", name="kvtp")
                nc.tensor.transpose(tp, kvblk[p], eye_sb)
                tps.append(tp)
            # all boundary evictions ride DVE: ACT must start the q(1)
            # eviction chain ASAP (it gates the q-psum recycle for q(2))
            kvts = []
            for p in range(MO):
                kvt = kvt_sb_pool.tile([P, P], HF, tag="kvts", name="kvts")
                nc.vector.tensor_copy(kvt, tps[p])
                kvts.append(kvt)
            for p in range(MO):
                kw = kvw_ps_pool.tile([P, 1024], FP, name="kvwp")
                for hh in range(2):
                    nc.tensor.matmul(
                        kw[:, hh * 512 : hh * 512 + F // 2],
                        lhsT=kvts[p],
                        rhs=wo_sb[:, p, hh * (F // 2) : (hh + 1) * (F // 2)],
                        start=True,
                        stop=True,
                    )
                nc.vector.tensor_copy(
                    kvwo_sb[:, p, :].rearrange("p (hh f) -> p hh f", hh=2),
                    kw.rearrange("p (hh f) -> p hh f", hh=2)[:, :, 0 : F // 2],
                )
        emit_q_mo(1, 2)

        # ------- stages B (Q.T), z (den+recip), qz, C2: staggered per chunk -
        with ExitStack() as sctx:
            zp_pool = sctx.enter_context(tc.tile_pool(name="zp", bufs=1, space="PSUM"))
            op_pool = sctx.enter_context(tc.tile_pool(name="outp", bufs=3, space="PSUM"))
            zr_pool = sctx.enter_context(tc.tile_pool(name="zrp", bufs=2))
            out_pool = sctx.enter_context(tc.tile_pool(name="osb", bufs=4))

            def emit_den(rc):
                # den_mo[c, n] = sum_g bmat[g, mo, c] q[g, mo, n]: the mask in
                # bmat makes every output partition c carry its own head's
                # denominator * 2^-12 — already expanded for the qz multiply.
                # recip costs ~3.3 cyc per FREE element (partition count is
                # free), so it stays per-mo [128, RC]; the qz multiplies run
                # on the B-phase-idle gpsimd to keep DVE off the c2 chain.
                rs = slice(rc * RC, (rc + 1) * RC)
                # zr must be fp32: reciprocal_approx_* depends on the fp32
                # bit layout, and a separate fp16 cast would cost more than
                # the 2x 16-bit qz mode saves
                zr = zr_pool.tile([P, MO, RC], FP, tag="zr", name="zr")
                for mo in range(MO):
                    zps = zp_pool.tile([P, RC], FP, tag=f"zps{mo}", name="zps")
                    nc.tensor.matmul(
                        zps, lhsT=bmat[:, mo, :], rhs=qt[:, mo, rs],
                        start=True, stop=True,
                    )
                    nc.vector.reciprocal_approx_fast(zr[:, mo, :], zps)
                    # qz rides z*2^12-scaled; the 2^-12 is folded into the C2
                    # output eviction. gpsimd's tensor_tensor is ~1.5us/op,
                    # so rc 0/1 (whose qz gates c2(0) right at the B start)
                    # stay on DVE; steady state keeps mo0 (the first block c2
                    # reads) on DVE and offloads mo1/mo2 to the idle gpsimd.
                    if rc < 2 or mo == 0:
                        eng = nc.vector
                    else:
                        eng = nc.gpsimd
                    eng.tensor_tensor(
                        qz[:, mo, rs], qt[:, mo, rs], zr[:, mo, :], op=ALU.mult
                    )

            def emit_c2(rc, subs=(0, 1, 2, 3)):
                # dense output projection, interleaved into the B loop so the
                # PE has fill work during the z latency chains
                for sub in subs:
                    r0 = rc * RC + sub * P
                    osb = out_pool.tile([P, F], HF)
                    for hh in range(2):
                        o_ps = op_pool.tile([P, F // 2], FP, tag="op", name="ops")
                        for p in range(MO):
                            nc.tensor.matmul(
                                o_ps,
                                lhsT=qz[:, p, r0 : r0 + P],
                                rhs=kvwo_sb[:, p, hh * (F // 2) : (hh + 1) * (F // 2)],
                                start=(p == 0),
                                stop=(p == MO - 1),
                            )
                        # output arrives 2^12-scaled (z pre-scale); exact
                        # unscale split across DVE and ACT; the two DMA halves
                        # ride the two HWDGE queues so the tail drains fast
                        oseg = osb[:, hh * (F // 2) : (hh + 1) * (F // 2)]
                        if hh == 0:
                            with nc.allow_low_precision(reason="fp16 output"):
                                nc.vector.tensor_scalar(
                                    oseg, o_ps, 1.0 / ZSCALE, None, op0=ALU.mult
                                )
                            if rc == NRC - 1:
                                # last chunk: fire each half's DMA immediately;
                                # the last two subs fan out across the
                                # tail-idle engines so the eviction->issue->
                                # transfer chains run in parallel
                                eng = (nc.sync, nc.sync, nc.gpsimd, nc.gpsimd)[sub]
                                eng.dma_start(y[r0 : r0 + P, 0 : F // 2], oseg)
                        else:
                            nc.scalar.activation(
                                oseg, o_ps, ACTF.Identity, scale=1.0 / ZSCALE
                            )
                            if rc == NRC - 1:
                                # no DMA issues on the ACT engine here: they
                                # would wedge between the hh1 evictions and
                                # delay the final eviction chain
                                eng = (nc.gpsimd, nc.gpsimd, nc.sync, nc.sync)[sub]
                                eng.dma_start(y[r0 : r0 + P, F // 2 :], oseg)
                    # one full-row DMA on the C2-idle sync queue (the ACT
                    # queue is near its limit with the hh=1 evictions)
                    if rc != NRC - 1:
                        nc.sync.dma_start(y[r0 : r0 + P, :], osb)

            # den(0) right at the B-stage start: its recip+qz chains (~5us on
            # gpsimd) drain under q(2) so c2(0) doesn't wait on them
            emit_den(0)
            # per iteration: den(rc-1) woven between q(rc)'s psum groups (its
            # qt landed last iteration; the zp psum was freed by the previous
            # recip, and the recip+qz chains drain under the remaining
            # matmuls), then c2(rc-2) (its qz completed a full iteration ago)
            # c2 sits early in the iteration so its psum-freeing evictions
            # land early in the DVE/ACT queues (the op-psum recycle is the
            # tightest cross-engine chain in phase B)
            for rc in range(2, NRC):
                emit_q_mo(rc, 0)
                emit_q_mo(rc, 1)
                emit_den(rc - 1)
                emit_q_mo(rc, 2)
                emit_c2(rc - 2)
            # tail: den(7)'s recip+qz chain drains under c2(6)'s matmuls
            emit_c2(NRC - 2, subs=(0,))
            emit_den(NRC - 1)
            emit_c2(NRC - 2, subs=(1, 2, 3))
            emit_c2(NRC - 1)

    nc.compile()
    return nc


def make_in_maps(x, Wq, bq, Wk, bk, Wv, bv, Wo, bo):
    """Shard full inputs into the 8 per-core input maps."""
    f32 = lambda a: np.ascontiguousarray(np.asarray(a, dtype=np.float32))
    f16 = lambda a: np.ascontiguousarray(np.asarray(a).astype(np.float16))
    # pre-block weights [in/out, cols] -> [partition, chunk, cols] so each
    # partition's DMA reads one contiguous run
    blkw = lambda a: np.ascontiguousarray(
        np.asarray(a).astype(np.float16).reshape(KO, P, G).transpose(1, 0, 2)
    )
    blko = lambda a: np.ascontiguousarray(
        np.asarray(a).astype(np.float16).reshape(MO, P, F).transpose(1, 0, 2)
    )
    # bv's output contribution is exactly the constant bv@Wo (z*(q.ksum)=1),
    # so it folds into the host-side bias add instead of riding the device
    _BO["bo_eff"] = np.asarray(
        np.asarray(bo, np.float64) + np.asarray(bv, np.float64) @ np.asarray(Wo, np.float64),
        np.float32,
    )
    Wq, Wk, Wv, Wo = map(f16, (Wq, Wk, Wv, Wo))
    bq = f32(bq)
    bk16 = f16(bk)
    x = np.asarray(x)
    # [768, 4096] -> [NCH, P, KO, P] so each 128-row block DMA reads
    # 1.5KB-contiguous runs
    xT = [
        np.ascontiguousarray(
            f16(x[b]).T.reshape(KO, P, NCH, P).transpose(2, 1, 0, 3)
        )
        for b in range(N_CORES // 2)
    ]
    # mask64[r, c] = 2^-12 where r and c fall in the same head-half; scaled by
    # ksum on-device it becomes the expanded z-denominator weight matrix
    mask64 = np.zeros((P, P), dtype=np.float16)
    mask64[:D, :D] = 1.0 / ZSCALE
    mask64[D:, D:] = 1.0 / ZSCALE
    eye = np.eye(P, dtype=np.float16)
    in_maps = []
    for c in range(N_CORES):
        b, g = divmod(c, 2)
        sl = slice(g * G, (g + 1) * G)
        in_maps.append(
            {
                "xt_in": xT[b],
                "wq": blkw(Wq[:, sl]),
                "wk": blkw(Wk[:, sl]),
                "wv": blkw(Wv[:, sl]),
                "wo": blko(Wo[sl, :]),
                "bq": f32(bq[sl]),
                "bkb": np.ascontiguousarray(np.tile(bk16[sl], (P, 1))),
                "mask64": mask64,
                "eye": eye,
            }
        )
    return in_maps


_BO = {}


def unshard(core_outs):
    """Sum the two row-parallel partials per batch element; add bo + bv@Wo."""
    bo_eff = _BO["bo_eff"]
    return np.stack(
        [
            core_outs[2 * b].astype(np.float32)
            + core_outs[2 * b + 1].astype(np.float32)
            + bo_eff
            for b in range(N_CORES // 2)
        ]
    )


_NC_CACHE = {}


def get_nc():
    if "nc" not in _NC_CACHE:
        _NC_CACHE["nc"] = build_nc()
    return _NC_CACHE["nc"]


def run(inputs, trace=False, **kwargs):
    nc = get_nc()
    in_maps = make_in_maps(**inputs)
    res = run_bass_kernel_spmd(
        nc, in_maps, core_ids=list(range(N_CORES)), trace=trace, **kwargs
    )
    out = unshard([r["y"] for r in res.results])
    return out, res


def kernel(**inputs):
    out, _ = run(inputs, trace=False)
    return out

